# revision 8
# baseline (speedup 1.0000x reference)
"""Trainium2 Bass kernel for nn_Discriminator_59442347376701.

Embedding lookup (one-hot matmul rewritten as a DMA gather) + bidirectional
LSTM + small MLP head, distributed over 8 NeuronCores as
(direction x batch-quarter).  Core c: direction = c//4 (0=fwd, 1=rev),
batch quarter g = c%4 (global sequences g*8 .. g*8+8).  Reverse cores get
time-reversed token indices from the host so the device program is uniform.

Layout: everything transposed -- hidden/gate dims on partitions, batch on
the free dim.  Per core the 8 sequences run as 2 staggered sub-chains of 4
so the cell-update latency of one hides under the matmuls of the other.

The head needs h_fwd and h_rev together: each pair {g, g+4} combines
partial W1 products with one small AllReduce, then forward core g emits
sigmoid(head) for its 8 sequences; the host concatenates 4x[8].
"""
import os
import sys

sys.path.insert(0, "/opt/trn_rl_repo")

import contextlib
import numpy as np
import ml_dtypes

import concourse.bass as bass
import concourse.tile as tile
from concourse import bacc, mybir
from concourse.bass_utils import run_bass_kernel_spmd

F32 = mybir.dt.float32
BF16 = mybir.dt.bfloat16
I32 = mybir.dt.int32
AF = mybir.ActivationFunctionType
ALU = mybir.AluOpType

VOCAB, EMB, H, LATENT, B, S = 50257, 128, 256, 64, 32, 128
G4 = 4 * H          # 1024 gate dims
NC = 8              # cores
BC = 8              # sequences per core
NSUB = 2            # staggered sub-chains per core
TOK = BC * S        # 1024 tokens per core
DBG = bool(int(os.environ.get("KDBG", "0")))
BF16NP = ml_dtypes.bfloat16


def _ap(base, layout):
    """Hand-built access pattern (for stride-0 broadcasts / reordered dims)."""
    return bass.AP(base.tensor, base.offset, layout)


def _emit(nc, tc, d):
    ctx = contextlib.ExitStack()
    with ctx:
        const = ctx.enter_context(tc.tile_pool(name="const", bufs=1))
        big = ctx.enter_context(tc.tile_pool(name="big", bufs=1))
        work = ctx.enter_context(tc.tile_pool(name="work", bufs=4))
        scan = ctx.enter_context(tc.tile_pool(name="scan", bufs=6))
        ps_scan = ctx.enter_context(tc.tile_pool(name="ps_scan", bufs=4, space="PSUM"))
        ps_xg = ctx.enter_context(tc.tile_pool(name="ps_xg", bufs=3, space="PSUM"))
        ps_tr = ctx.enter_context(tc.tile_pool(name="ps_tr", bufs=1, space="PSUM"))

        def load(name, shape, dt):
            t = const.tile(list(shape), dt, tag=name)
            nc.sync.dma_start(t[:], d[name][:])
            return t

        idx = load("idx", (128, BC), I32)
        idxa = load("idxa", (BC, 1), I32)
        whhT = load("whhT", (128, 2 * G4), BF16)
        wihT = load("wihT", (128, G4), BF16)
        bvec = load("bvec", (128, 8), F32)
        idf = load("identf", (128, 128), F32)
        idb = load("identb", (128, 128), BF16)
        onesb = load("onesb", (128, 128), BF16)
        c0c1 = load("c0c1", (128, 2), F32)
        w1ta = load("W1TA", (128, 512), BF16)
        w1tb = load("W1TB", (128, 256), BF16)
        w2t = load("W2T", (128, 128), BF16)
        wdt = load("WdT", (64, 1), BF16)
        b1c = load("b1c", (128, 2), F32)
        b2c = load("b2c", (64, 1), F32)
        bdc = load("bdc", (1, 1), F32)
        al0 = load("al0", (128, 1), F32)
        al1 = load("al1", (128, 1), F32)

        # ---- embedding gather: token n = m*128+p -> g_nat[p, m*128:(m+1)*128] ----
        g_nat = big.tile([128, TOK], F32, tag="g_nat")
        nc.gpsimd.indirect_dma_start(
            out=g_nat[:], out_offset=None,
            in_=d["W_emb"][:],
            in_offset=bass.IndirectOffsetOnAxis(ap=idx[:], axis=0))
        g_a = work.tile([BC, 128], F32, tag="g_a")
        nc.gpsimd.indirect_dma_start(
            out=g_a[:], out_offset=None,
            in_=d["W_emb"][:],
            in_offset=bass.IndirectOffsetOnAxis(ap=idxa[:], axis=0))

        # ---- transpose blocks -> embT [128, 1024] bf16, col n = t*8 + j ----
        embT = big.tile([128, TOK], BF16, tag="embT")
        for m in range(8):
            pt = ps_tr.tile([128, 128], F32, tag="ps_tr")
            nc.tensor.transpose(pt[:], g_nat[:, m * 128:(m + 1) * 128], idf[:])
            if m % 2 == 0:
                nc.vector.tensor_copy(embT[:, m * 128:(m + 1) * 128], pt[:])
            else:
                nc.scalar.copy(embT[:, m * 128:(m + 1) * 128], pt[:])

        pa = ps_tr.tile([128, 128], F32, tag="ps_tr")
        nc.tensor.transpose(pa[:, 0:BC], g_a[:], idf[0:BC, 0:BC])
        embaT = const.tile([128, BC], F32, tag="embaT")
        nc.vector.tensor_copy(embaT[:], pa[:, 0:BC])

        # ---- xg projection -> xg [128, S*64] bf16; col = t*64 + sub*32 + mc*4 + b ----
        xg = big.tile([128, S * 64], BF16, tag="xg")
        for half in range(2):
            for mc in range(8):
                pxg = ps_xg.tile([128, 512], F32, tag="ps_xg")
                nc.tensor.matmul(
                    pxg[:], lhsT=wihT[:, mc * 128:(mc + 1) * 128],
                    rhs=embT[:, half * 512:(half + 1) * 512],
                    start=True, stop=True)
                rd = pxg[:].rearrange("p (t s b) -> p t s b", s=2, b=4)
                wr = _ap(xg[:, half * 4096 + mc * 4],
                         [[S * 64, 128], [64, 64], [32, 2], [1, 4]])
                bmc = bvec[:, mc:mc + 1]
                if mc % 2 == 0:
                    nc.vector.tensor_scalar(wr, rd, bmc, None, op0=ALU.add)
                else:
                    nc.scalar.activation(wr, rd, AF.Identity, bias=bmc, scale=1.0)

        # ---- lengths + latch masks ----
        nz = work.tile([128, BC], BF16, tag="nz")
        nc.vector.tensor_scalar(nz[:], idx[:], 0, None, op0=ALU.not_equal)
        pcount = ps_scan.tile([128, BC], F32, tag="ps_scan")
        nc.tensor.matmul(pcount[:], lhsT=onesb[:], rhs=nz[:], start=True, stop=True)
        Lt = work.tile([128, BC], F32, tag="Lt")
        nc.vector.tensor_scalar_max(Lt[:], pcount[:], 1.0)
        qt = const.tile([128, BC], F32, tag="qt")
        c0b = _ap(c0c1[:, 0:1], [[2, 128], [0, BC]])
        nc.vector.scalar_tensor_tensor(
            qt[:], Lt[:], c0c1[:, 1:2], c0b, op0=ALU.mult, op1=ALU.add)

        ioi = big.tile([128, 8 * S], I32, tag="ioi")
        nc.gpsimd.iota(ioi[:], pattern=[[0, 8], [1, S]], base=0, channel_multiplier=0)
        iof = big.tile([128, 8 * S], F32, tag="iof")
        nc.vector.tensor_copy(iof[:], ioi[:])
        masks = []
        for sub in range(NSUB):
            mk = big.tile([128, 8 * S], BF16, tag=f"mask{sub}")
            qv = _ap(qt[:, sub * 4], [[BC, 128], [0, 2], [1, 4], [0, S]])
            nc.vector.tensor_tensor(
                mk[:].rearrange("p (ch b t) -> p ch b t", ch=2, b=4),
                iof[:].rearrange("p (ch b t) -> p ch b t", ch=2, b=4),
                qv, op=ALU.is_equal)
            masks.append(mk)

        # ---- LSTM scan: 128 steps x 2 staggered sub-chains ----
        hist = [big.tile([128, S * 8], BF16, tag=f"hist{s}", name=f"hist{s}")
                for s in range(NSUB)]
        hinit = [const.tile([128, 8], BF16, tag=f"hinit{s}", name=f"hinit{s}")
                 for s in range(NSUB)]
        ctile = [const.tile([128, 8], F32, tag=f"c{s}", name=f"c{s}")
                 for s in range(NSUB)]
        for s in range(NSUB):
            nc.vector.memset(hinit[s][:], 0)
            nc.vector.memset(ctile[s][:], 0)

        for st in range(S):
            for sub in range(NSUB):
                ps = ps_scan.tile([128, 32], F32, tag="ps_scan")
                nc.tensor.matmul(
                    ps[:], lhsT=idb[:],
                    rhs=xg[:, st * 64 + sub * 32: st * 64 + sub * 32 + 32],
                    start=True, stop=False, skip_group_check=True)
                hprev = hinit[sub][:] if st == 0 else hist[sub][:, (st - 1) * 8: st * 8]
                for k in range(2):
                    for mc in range(8):
                        nc.tensor.matmul(
                            ps[:, mc * 4:(mc + 1) * 4],
                            lhsT=whhT[:, k * G4 + mc * 128: k * G4 + (mc + 1) * 128],
                            rhs=hprev[:, k * 4:(k + 1) * 4],
                            start=False, stop=(k == 1 and mc == 7),
                            skip_group_check=True)
                # all 4 gates with one sigmoid; g-gate rows were pre-scaled x2
                # on the host so tanh(x) = 2*sigmoid(2x) - 1 applies.
                sg = scan.tile([128, 32], F32, tag=f"sg{sub}")
                nc.scalar.activation(sg[:], ps[:], AF.Sigmoid)
                i_, f_, g_, o_ = (sg[:, 0:8], sg[:, 8:16], sg[:, 16:24], sg[:, 24:32])
                c = ctile[sub]
                t1 = scan.tile([128, 8], F32, tag=f"t1{sub}")
                nc.vector.tensor_mul(t1[:], i_, g_)
                t2 = scan.tile([128, 8], F32, tag=f"t2{sub}")
                nc.vector.tensor_mul(t2[:], f_, c[:])
                t3 = scan.tile([128, 8], F32, tag=f"t3{sub}")
                nc.vector.scalar_tensor_tensor(t3[:], i_, -1.0, t2[:],
                                               op0=ALU.mult, op1=ALU.add)
                nc.vector.scalar_tensor_tensor(c[:], t1[:], 2.0, t3[:],
                                               op0=ALU.mult, op1=ALU.add)
                s2 = scan.tile([128, 8], F32, tag=f"s2{sub}")
                nc.scalar.activation(s2[:], c[:], AF.Sigmoid, scale=2.0)
                t4 = scan.tile([128, 8], F32, tag=f"t4{sub}")
                nc.vector.tensor_mul(t4[:], o_, s2[:])
                hcur = hist[sub][:, st * 8:(st + 1) * 8]
                nc.vector.scalar_tensor_tensor(hcur, t4[:], 2.0, o_,
                                               op0=ALU.mult, op1=ALU.subtract)

        # ---- latch h at t = lengths-1 (fwd) / 128-lengths (rev step index) ----
        last = const.tile([128, 2 * BC], F32, tag="last")
        for sub in range(NSUB):
            tmp = big.tile([128, 8 * S], F32, tag="latchtmp")
            nc.vector.tensor_tensor(
                tmp[:].rearrange("p (c t) -> p c t", c=8),
                hist[sub][:].rearrange("p (t c) -> p c t", c=8),
                masks[sub][:].rearrange("p (c t) -> p c t", c=8),
                op=ALU.mult)
            nc.vector.tensor_reduce(
                last[:, sub * 8:(sub + 1) * 8],
                tmp[:].rearrange("p (c t) -> p c t", c=8),
                axis=mybir.AxisListType.X, op=ALU.add)

        # ---- head ----
        def prelu(dst, src, alpha_ap):
            pos = work.tile(list(src.shape), F32, tag="prelu_pos")
            neg = work.tile(list(src.shape), F32, tag="prelu_neg")
            nc.vector.tensor_scalar_max(pos[:], src, 0.0)
            nc.vector.tensor_scalar_min(neg[:], src, 0.0)
            nc.vector.scalar_tensor_tensor(dst, neg[:], alpha_ap, pos[:],
                                           op0=ALU.mult, op1=ALU.add)

        pll = const.tile([128, 2 * BC], BF16, tag="pll")
        prelu(pll[:], last[:], al0[:, 0:1])
        plea = const.tile([128, BC], BF16, tag="plea")
        prelu(plea[:], embaT[:], al0[:, 0:1])

        # partial W1 product for own 8 sequences: px [128, 16] (m*8 + b)
        px = const.tile([128, 16], F32, tag="px")
        for m in range(2):
            pp = ps_scan.tile([128, BC], F32, tag="ps_scan")
            for k in range(2):
                rhs = _ap(pll[:, k * 4], [[2 * BC, 128], [8, 2], [1, 4]])
                nc.tensor.matmul(
                    pp[:], lhsT=w1ta[:, k * 256 + m * 128: k * 256 + (m + 1) * 128],
                    rhs=rhs, start=(k == 0), stop=False, skip_group_check=True)
            nc.tensor.matmul(pp[:], lhsT=w1tb[:, m * 128:(m + 1) * 128], rhs=plea[:],
                             start=False, stop=True, skip_group_check=True)
            nc.vector.tensor_copy(px[:, m * 8:(m + 1) * 8], pp[:])
        nc.sync.dma_start(d["partial"][:], px[:])
        nc.gpsimd.collective_compute(
            "AllReduce", ALU.add,
            replica_groups=[[0, 4], [1, 5], [2, 6], [3, 7]],
            ins=[d["partial"][:]], outs=[d["arshared"][:]])
        arx = const.tile([128, 16], F32, tag="arx")
        nc.sync.dma_start(arx[:], d["arshared"][:])

        x1 = const.tile([128, 16], BF16, tag="x1")
        for m in range(2):
            xb = work.tile([128, 8], F32, tag="xb")
            nc.vector.tensor_scalar(xb[:], arx[:, m * 8:(m + 1) * 8],
                                    b1c[:, m:m + 1], None, op0=ALU.add)
            prelu(x1[:, m * 8:(m + 1) * 8], xb[:], al1[:, 0:1])
        p2 = ps_scan.tile([64, BC], F32, tag="ps_scan")
        for k in range(2):
            nc.tensor.matmul(p2[:], lhsT=w2t[:, k * 64:(k + 1) * 64],
                             rhs=x1[:, k * 8:(k + 1) * 8],
                             start=(k == 0), stop=(k == 1), skip_group_check=True)
        x2 = const.tile([64, BC], BF16, tag="x2")
        nc.scalar.activation(x2[:], p2[:], AF.Identity, bias=b2c[:, 0:1])
        pd = ps_scan.tile([1, BC], F32, tag="ps_scan")
        nc.tensor.matmul(pd[:], lhsT=wdt[:], rhs=x2[:], start=True, stop=True,
                         skip_group_check=True)
        outs = const.tile([1, BC], F32, tag="outs")
        nc.scalar.activation(outs[:], pd[:], AF.Sigmoid, bias=bdc[:, 0:1])
        nc.sync.dma_start(d["out"][:], outs[:])

        if DBG:
            nc.sync.dma_start(d["dbg_q"][:], qt[:])
            nc.sync.dma_start(d["dbg_last"][:], last[:])
            nc.sync.dma_start(d["dbg_px"][:], px[:])
            nc.sync.dma_start(d["dbg_embT"][:], embT[:])
            nc.sync.dma_start(d["dbg_xg"][:], xg[:])
            nc.sync.dma_start(d["dbg_hist0"][:], hist[0][:])


_CACHE = {}

_IN_SPECS = [
    ("W_emb", (VOCAB, EMB), F32), ("idx", (128, BC), I32), ("idxa", (BC, 1), I32),
    ("whhT", (128, 2 * G4), BF16), ("wihT", (128, G4), BF16), ("bvec", (128, 8), F32),
    ("identf", (128, 128), F32), ("identb", (128, 128), BF16), ("onesb", (128, 128), BF16),
    ("c0c1", (128, 2), F32), ("W1TA", (128, 512), BF16), ("W1TB", (128, 256), BF16),
    ("W2T", (128, 128), BF16), ("WdT", (64, 1), BF16), ("b1c", (128, 2), F32),
    ("b2c", (64, 1), F32), ("bdc", (1, 1), F32), ("al0", (128, 1), F32), ("al1", (128, 1), F32),
]


def _build():
    if "nc" in _CACHE:
        return _CACHE["nc"]
    nc = bacc.Bacc("TRN2", target_bir_lowering=False, debug=False, num_devices=NC)
    d = {}
    for name, shape, dt in _IN_SPECS:
        d[name] = nc.dram_tensor(name, shape, dt, kind="ExternalInput").ap()
    d["out"] = nc.dram_tensor("out", (1, BC), F32, kind="ExternalOutput").ap()
    d["partial"] = nc.dram_tensor("partial", (128, 16), F32, kind="Internal").ap()
    d["arshared"] = nc.dram_tensor("arshared", (128, 16), F32, kind="Internal").ap()
    if DBG:
        for nm, shape in [("dbg_q", (128, BC)), ("dbg_last", (128, 16)),
                          ("dbg_px", (128, 16))]:
            d[nm] = nc.dram_tensor(nm, shape, F32, kind="ExternalOutput").ap()
        for nm, shape in [("dbg_embT", (128, TOK)), ("dbg_xg", (128, S * 64)),
                          ("dbg_hist0", (128, S * 8))]:
            d[nm] = nc.dram_tensor(nm, shape, BF16, kind="ExternalOutput").ap()

    with tile.TileContext(nc) as tc:
        _emit(nc, tc, d)
    nc.compile()
    _CACHE["nc"] = nc
    return nc


def _prep_core_inputs(s, a, W_emb, w_ih_f, w_hh_f, b_f, w_ih_r, w_hh_r, b_r,
                      alpha0, alpha1, W1, b1, W2, b2, Wd, bd):
    """Host-side sharding / weight preprocessing -> list of 8 in_maps."""
    def eff(w_ih, w_hh, bb):
        wi = w_ih.astype(np.float64).copy()
        wh = w_hh.astype(np.float64).copy()
        be = bb.astype(np.float64).copy()
        wi[512:768] *= 2.0   # g-gate rows: tanh(x) = 2*sigmoid(2x) - 1
        wh[512:768] *= 2.0
        be[512:768] *= 2.0
        # whhT [128, 2*G4]: col k*G4 + gd  <-  w_hh.T[k*128+p, gd]
        whhT = np.empty((128, 2 * G4), np.float64)
        for k in range(2):
            whhT[:, k * G4:(k + 1) * G4] = wh[:, k * 128:(k + 1) * 128].T
        wihT = wi.T  # [128, 1024]
        bvec = be.reshape(8, 128).T.copy()  # bvec[p, mc] = be[mc*128+p]
        return (whhT.astype(BF16NP), wihT.astype(BF16NP), bvec.astype(np.float32))

    whhT_f, wihT_f, bvec_f = eff(w_ih_f, w_hh_f, b_f)
    whhT_r, wihT_r, bvec_r = eff(w_ih_r, w_hh_r, b_r)

    # W1TA fwd: W1 cols 0:256 (h_f part); rev: W1 cols 256:512 (h_r part)
    def w1ta_for(col0):
        out = np.empty((128, 512), np.float32)
        for k in range(2):
            for m in range(2):
                blk = W1[m * 128:(m + 1) * 128, col0 + k * 128: col0 + (k + 1) * 128]
                out[:, k * 256 + m * 128: k * 256 + (m + 1) * 128] = blk.T
        return out.astype(BF16NP)

    w1ta_f = w1ta_for(0)
    w1ta_r = w1ta_for(256)
    w1tb_f = np.empty((128, 256), np.float32)
    for m in range(2):
        w1tb_f[:, m * 128:(m + 1) * 128] = W1[m * 128:(m + 1) * 128, 512:640].T
    w1tb_f = w1tb_f.astype(BF16NP)
    w1tb_r = np.zeros((128, 256), BF16NP)

    w2t = np.empty((128, 128), np.float32)
    for k in range(2):
        w2t[:, k * 64:(k + 1) * 64] = W2[:, k * 128:(k + 1) * 128].T
    w2t = w2t.astype(BF16NP)
    wdt = Wd.T.astype(BF16NP)                      # [64, 1]
    b1c = b1.reshape(2, 128).T.astype(np.float32)  # [128, 2]
    b2c = b2.reshape(64, 1).astype(np.float32)
    bdc = bd.reshape(1, 1).astype(np.float32)
    al0 = np.full((128, 1), float(np.asarray(alpha0).ravel()[0]), np.float32)
    al1 = np.full((128, 1), float(np.asarray(alpha1).ravel()[0]), np.float32)
    identf = np.eye(128, dtype=np.float32)
    identb = np.eye(128, dtype=np.float32).astype(BF16NP)
    onesb = np.ones((128, 128), np.float32).astype(BF16NP)
    W_emb32 = np.ascontiguousarray(W_emb.astype(np.float32))
    s = np.asarray(s).astype(np.int64)
    a = np.asarray(a).astype(np.int64)

    in_maps = []
    for c in range(NC):
        rev = c >= 4
        g = c % 4
        sg = s[g * 8:(g + 1) * 8]                  # [8, S]
        st = sg[:, ::-1] if rev else sg            # time order for this core
        # idx[p, m]: token n = m*128 + p ; (t, j) = (n//8, n%8)
        n = (np.arange(8)[None, :] * 128 + np.arange(128)[:, None])  # [128, 8]
        t_of = n // 8
        j_of = n % 8
        idxv = st[j_of, t_of].astype(np.int32)
        idxa = a[g * 8:(g + 1) * 8].astype(np.int32).reshape(BC, 1)
        c0 = 128.0 if rev else -1.0
        c1 = -1.0 if rev else 1.0
        c0c1 = np.tile(np.array([[c0, c1]], np.float32), (128, 1))
        in_maps.append({
            "W_emb": W_emb32, "idx": idxv, "idxa": idxa,
            "whhT": whhT_r if rev else whhT_f,
            "wihT": wihT_r if rev else wihT_f,
            "bvec": bvec_r if rev else bvec_f,
            "identf": identf, "identb": identb, "onesb": onesb,
            "c0c1": c0c1,
            "W1TA": w1ta_r if rev else w1ta_f,
            "W1TB": w1tb_r if rev else w1tb_f,
            "W2T": w2t, "WdT": wdt, "b1c": b1c, "b2c": b2c, "bdc": bdc,
            "al0": al0, "al1": al1,
        })
    return in_maps


def kernel(**inputs):
    inputs = {k: np.asarray(v) for k, v in inputs.items()}
    nc = _build()
    in_maps = _prep_core_inputs(**inputs)
    kwargs = {}
    if os.environ.get("KTRACE"):
        kwargs = dict(trace=True, trace_cores=list(range(NC)))
    res = run_bass_kernel_spmd(nc, in_maps, core_ids=list(range(NC)), **kwargs)
    _CACHE["last_results"] = res
    out = np.concatenate([res.results[g]["out"].reshape(BC) for g in range(4)])
    return out.reshape(B, 1).astype(np.float32)
